# revision 13
# baseline (speedup 1.0000x reference)
"""Sharded retrieval-KNN kernel for Trainium2 (8 NeuronCores).

Self-contained: kernel(**inputs) -> np.ndarray [64, 64].

v4 design (v1 ~103us, v3 ~99us; op-level rates hardware-measured):
 - fp8 e4m3 code stream (codes centered by -128; attention weight cast
   directly) fed straight to the PE (fp8 moving x f16 stationary), so
   ACT does NO dtype conversion. Host re-score is bit-exact; fp8 only
   perturbs CANDIDATE selection (validated margin: worst bucket rank 2
   of the top-8-per-window budget).
 - whole code table SBUF-resident (61KB/partition); all stream DMAs
   issued up-front so PE never waits on buffer recycling.
 - PSUM drain split across both PSUM-capable engines at measured rates
   (DVE 1.042ns/elem + 130ns/instr; ACT copy 0.833ns/elem + 293ns;
   DVE f16 tensor_tensor max in 2x_1p mode 0.521ns/out; tensor_reduce
   has NO fast mode):
     A-chunks: DVE tensor_reduce max PSUM->pm (consecutive 64-col
       buckets);
     B-chunks: ACT copies PSUM->f16 staging; regions of 8/4/2/1
       B-chunks (not necessarily source-consecutive) fold INCREMENTALLY
       on DVE: each pair of staged chunks is folded as soon as copied
       (binary-counter merge), then the final 2048-wide running max is
       halved down to the region's nb buckets. Region buckets are the
       mod-nb staged-position classes, so every fold distance (2048,
       1024, ..., nb) preserves bucket membership; power-of-2 region
       sizes make chunk-pairwise folding legal.
 - A-chunks sit every ~4-5 chunks so DVE has filler while ACT copies,
   keeping the 2-slot PSUM recirculation from serializing; the stream
   ends on A-work + a tiny window-2 scan instead of a deep tree.
 - DVE Max8 + MaxIndex8 over 2 windows (704/273 buckets); [128, 8] u32
   bucket indices DMA'd out per window as soon as its scan finishes.

Host glue: exact fp32 re-score of expanded candidates (16 buckets x
64 slots x 2 parities x 8 cores per query), global top-k, softmax,
value projection.
"""

import sys
sys.path.insert(0, '/opt/trn_rl_repo')

import numpy as np
import ml_dtypes
import concourse.bass as bass
import concourse.mybir as mybir
from concourse import bacc, tile

F16 = mybir.dt.float16
F32 = mybir.dt.float32
F8 = mybir.dt.float8e4
U8 = mybir.dt.uint8
U32 = mybir.dt.uint32
ALU = mybir.AluOpType
AX = mybir.AxisListType

D = 64             # embedding dim
B = 64             # queries
NCORES = 8
N = 1_000_000
NSH = N // NCORES  # 125000 slots per core
NP = 125056        # padded slots per core (56 pad slots)
LANE = NP // 2     # 62528 per-parity lane columns
FOLD = 64          # cols per bucket
NB = LANE // FOLD  # 977 buckets per lane
NTOP = 8
NW = 2
NCAND = NW * NTOP

CW = 1280                      # chunk width; 3 PSUM slots of 1280
WIDTHS = [CW] * 48 + [1088]
NCH = 49
A_SET = frozenset(set(range(0, 48, 4)) | {48})
REG_SIZES = [8, 8, 8, 8, 4]    # B-chunk regions, in order of appearance
DMA_SPANS = [(0, 1280), (1280, 1280), (2560, 2560), (5120, 5120),
             (10240, 7680), (17920, 7680), (25600, 7680), (33280, 7680),
             (40960, 7680), (48640, 7680), (56320, 5120), (61440, 1088)]
ORDER = list(range(49))
assert sum(l for _, l in DMA_SPANS) == LANE and sum(WIDTHS) == LANE
B_LIST = [i for i in range(NCH) if i not in A_SET]
assert sum(REG_SIZES) == len(B_LIST)
REGIONS = []
_i = 0
for _s in REG_SIZES:
    REGIONS.append(B_LIST[_i:_i + _s])
    _i += _s
for _r in REGIONS:
    assert all(WIDTHS[c] == CW for c in _r), 'region chunks must be 2048'

# segment layout in COMPLETION order (A-chunk at its index; region at its
# last chunk): (kind, payload, q0, nb)
def _layout():
    segs = []
    q0 = 0
    done_at = []
    g0s = np.concatenate([[0], np.cumsum(WIDTHS)]).astype(np.int64)
    for ci in sorted(A_SET):
        done_at.append((ORDER.index(ci), ('A', ci)))
    for ri, r in enumerate(REGIONS):
        done_at.append((max(ORDER.index(c) for c in r), ('B', ri)))
    done_at.sort()
    for _, (kind, x) in done_at:
        if kind == 'A':
            nb = WIDTHS[x] // FOLD
            segs.append(('A', x, q0, nb))
        else:
            nb = sum(WIDTHS[c] for c in REGIONS[x]) // FOLD
            segs.append(('B', x, q0, nb))
        q0 += nb
    assert q0 == NB
    return segs, g0s


SEGS, G0S = _layout()
_w1 = max(q0 + nb for k, x, q0, nb in SEGS if q0 + nb <= 720)
WIN = (_w1, NB - _w1)
Q0_OF_A = {x: q0 for k, x, q0, nb in SEGS if k == 'A'}
Q0_OF_R = {x: q0 for k, x, q0, nb in SEGS if k == 'B'}


def bucket_columns():
    """[NB, FOLD] lane-column indices of each global bucket."""
    cols = np.zeros((NB, FOLD), np.int64)
    for kind, x, q0, nb in SEGS:
        if kind == 'A':
            g0 = G0S[x]
            for b in range(nb):
                cols[q0 + b] = g0 + FOLD * b + np.arange(FOLD)
        else:
            r = REGIONS[x]
            s = np.arange(len(r) * CW)
            piece = s // CW
            col = G0S[np.asarray(r)[piece]] + (s % CW)
            for b in range(nb):
                cols[q0 + b] = col[s % nb == b]
    return cols


def build_kernel():
    nc = bacc.Bacc("TRN2", target_bir_lowering=False, debug=False,
                   num_devices=NCORES)

    codesT = nc.dram_tensor('codesT', [128, LANE + 256], U8,
                            kind='ExternalInput')
    o_idx = nc.dram_tensor('o_idx', [128, NCAND], U32, kind='ExternalOutput')

    wb = [0, WIN[0], NB]

    with tile.TileContext(nc) as tc:
        with tc.tile_pool(name='persist', bufs=1) as pp:
            lhsT_raw = pp.tile([128, 256], U8)
            nc.scalar.dma_start(lhsT_raw[:, :], codesT[:, LANE:LANE + 256])
            lhsT_sb = lhsT_raw[:, :].bitcast(F16)

            codes = pp.tile([128, LANE], U8, tag='codes')
            for off, span in DMA_SPANS:
                nc.sync.dma_start(codes[:, off:off + span],
                                  codesT[:, off:off + span])

            pm = pp.tile([128, NB], F16, tag='pm')
            wmax = pp.tile([128, NCAND], F16, tag='wmax')
            widx = pp.tile([128, NCAND], U32, tag='widx')
            # incremental-fold scratch: pair outputs, merge ping-pong,
            # halving ladder (all f16, offsets 4B-aligned)
            scrP = pp.tile([128, 4096], F16, tag='scrP')
            scrM = pp.tile([128, 4096], F16, tag='scrM')
            scrQ = pp.tile([128, 2048], F16, tag='scrQ')
            scrH = pp.tile([128, 1024], F16, tag='scrH')
            scrH2 = pp.tile([128, 512], F16, tag='scrH2')

            done_q = [0]

            def scan_windows(done, prev):
                for w in range(NW):
                    if done >= wb[w + 1] and prev < wb[w + 1]:
                        nc.vector.max(out=wmax[:, w * 8:(w + 1) * 8],
                                      in_=pm[:, wb[w]:wb[w + 1]])
                        nc.vector.max_index(
                            out=widx[:, w * 8:(w + 1) * 8],
                            in_max=wmax[:, w * 8:(w + 1) * 8],
                            in_values=pm[:, wb[w]:wb[w + 1]])
                        nc.sync.dma_start(
                            o_idx[:, w * 8:(w + 1) * 8],
                            widx[:, w * 8:(w + 1) * 8])

            def add_buckets(nb):
                prev = done_q[0]
                done_q[0] = prev + nb
                scan_windows(done_q[0], prev)

            with tc.tile_pool(name='stage', bufs=2) as sp, \
                 tc.tile_pool(name='ps', bufs=1, space='PSUM') as xp:
                ps = xp.tile([128, 3840], F32)

                reg_of = {}
                for ri, r in enumerate(REGIONS):
                    for c in r:
                        reg_of[c] = ri
                rstate = {}        # ri -> dict(st, n, stack, n2, n4)

                def emit_merges(ri):
                    """binary-counter merges; stack holds (ap, vchunks)
                    pieces, each 2048 wide. v2 pieces live in scrP, v4 in
                    scrM (2 slots), v8 in scrQ — a merge never writes the
                    buffer it reads."""
                    st = rstate[ri]
                    stack = st['stack']
                    while len(stack) >= 2 and stack[-1][1] == stack[-2][1]:
                        bsrc, v = stack.pop()
                        asrc, _ = stack.pop()
                        if v == 1:
                            dst = scrP[:, (st['n2'] % 2) * CW:
                                       (st['n2'] % 2 + 1) * CW]
                            st['n2'] += 1
                        elif v == 2:
                            dst = scrM[:, (st['n4'] % 2) * CW:
                                       (st['n4'] % 2 + 1) * CW]
                            st['n4'] += 1
                        else:
                            dst = scrQ[:, :CW]
                        nc.vector.tensor_tensor(dst, asrc, bsrc, ALU.max)
                        stack.append((dst, v * 2))

                def finish_region(ri):
                    st = rstate[ri]
                    assert len(st['stack']) == 1, 'region size must be 2^k'
                    cur, _ = st['stack'].pop()
                    nb = sum(WIDTHS[c] for c in REGIONS[ri]) // FOLD
                    cw = CW
                    ladder = (scrH, scrH2)
                    li = 0
                    while cw // 2 > nb:
                        half = cw // 2
                        dst = ladder[li % 2][:, :half]
                        nc.vector.tensor_tensor(dst, cur[:, :half],
                                                cur[:, half:cw], ALU.max)
                        cur, cw = dst, half
                        li += 1
                    q0 = Q0_OF_R[ri]
                    nc.vector.tensor_tensor(pm[:, q0:q0 + nb], cur[:, :nb],
                                            cur[:, nb:cw], ALU.max)
                    add_buckets(nb)

                for pi, ci in enumerate(ORDER):
                    gw = WIDTHS[ci]
                    g0 = int(G0S[ci])
                    poff = (pi % 3) * CW
                    r0 = 0
                    while r0 < gw:
                        # MULT output must not cross a PSUM 4-bank group
                        # boundary (2048 fp32)
                        nxt = 2048 - ((poff + r0) % 2048)
                        bw = min(512, gw - r0, nxt)
                        nc.tensor.matmul(
                            ps[:, poff + r0:poff + r0 + bw], lhsT_sb,
                            codes[:, g0 + r0:g0 + r0 + bw].bitcast(F8),
                            start=True, stop=True)
                        r0 += bw
                    if ci in A_SET:
                        q0 = Q0_OF_A[ci]
                        nc.vector.tensor_reduce(
                            pm[:, q0:q0 + gw // FOLD],
                            ps[:, poff:poff + gw].rearrange(
                                'p (q k) -> p q k', k=FOLD),
                            AX.X, ALU.max)
                        add_buckets(gw // FOLD)
                    else:
                        ri = reg_of[ci]
                        if ri not in rstate:
                            stt = sp.tile([128, 8 * CW], F16, tag='st')
                            rstate[ri] = dict(st=stt, n=0, stack=[],
                                              n2=0, n4=0)
                        st = rstate[ri]
                        soff = st['n'] * CW
                        nc.scalar.copy(st['st'][:, soff:soff + CW],
                                       ps[:, poff:poff + CW])
                        st['n'] += 1
                        st['stack'].append(
                            (st['st'][:, soff:soff + CW], 1))
                        emit_merges(ri)
                        if ci == REGIONS[ri][-1]:
                            finish_region(ri)
    return nc


# ---------------- host glue ----------------

def _quant_params(memory):
    mn = memory.min()
    mx = memory.max()
    scale = (mx - mn) / np.float32(255.0)
    zp = -mn / scale
    return np.float32(scale), np.float32(zp)


def prep_inputs(query, memory, attention_weights, Wq, Wk, Wv):
    scale, zp = _quant_params(memory)
    codes = np.rint(memory / scale + zp).astype(np.float32)    # [N, 64]
    v8 = (codes[:, :63] - np.float32(128.0)).astype(
        ml_dtypes.float8_e4m3)                                 # [N, 63]
    aw8 = attention_weights.astype(ml_dtypes.float8_e4m3)      # [N]

    q = query @ Wq.T
    qk = (q @ Wk) / np.float32(np.sqrt(D))                     # [B, D]
    qks16 = (scale * qk[:, :63]).astype(np.float16)            # [B, 63]
    L = np.zeros((128, 128), np.float16)
    L[0:63, 0:64] = qks16.T
    L[63, 0:64] = np.float16(1.0)
    L[64:127, 64:128] = qks16.T
    L[127, 64:128] = np.float16(1.0)

    in_maps = []
    for c in range(NCORES):
        r64 = np.zeros((NP, 64), np.uint8)
        r64[:NSH, :63] = v8[c * NSH:(c + 1) * NSH].view(np.uint8)
        r64[:NSH, 63] = aw8[c * NSH:(c + 1) * NSH].view(np.uint8)
        codesT_h = np.ascontiguousarray(np.concatenate(
            [r64.reshape(LANE, 2, 64).transpose(1, 2, 0).reshape(128, LANE),
             L.view(np.uint8)], axis=1))
        in_maps.append(dict(codesT=codesT_h))
    return in_maps, scale, zp, qk


def host_tail(results, memory, attention_weights, Wv, scale, zp, qk, top_k):
    aw = attention_weights
    wb = [0, WIN[0], NB]
    bcols = bucket_columns()                                   # [NB, 64]
    cand = [[] for _ in range(B)]
    for c, r in enumerate(results):
        widx = r['o_idx'].astype(np.int64)                     # [128, 16]
        for p in range(128):
            par = 1 if p >= 64 else 0
            q_ = p % 64
            buckets = np.concatenate(
                [widx[p, w * 8:(w + 1) * 8] + wb[w] for w in range(NW)])
            cols = bcols[buckets].ravel()
            sl = 2 * cols + par
            ok = sl < NSH
            if ok.any():
                cand[q_].extend((c * NSH + sl[ok]).tolist())
    out = np.zeros((B, D), np.float32)
    for b in range(B):
        cs = np.unique(np.array(cand[b], dtype=np.int64))
        mdq = (np.rint(memory[cs] / scale + zp) - zp) * scale
        ss = qk[b] @ mdq.T + aw[cs]
        k = min(int(top_k), len(cs))
        ti = np.argsort(-ss, kind='stable')[:k]
        ts = ss[ti]
        w_ = np.exp(ts - ts.max())
        w_ = (w_ / w_.sum()).astype(np.float32)
        vals = mdq[ti] @ Wv.T
        out[b] = w_ @ vals
    return out


# ---------------- PJRT runner ----------------

import jax
from jax.sharding import Mesh, PartitionSpec
from jax.experimental.shard_map import shard_map
from concourse import bass2jax


def make_runner(nc, n_cores=8):
    bass2jax.install_neuronx_cc_hook()
    partition_name = nc.partition_id_tensor.name if nc.partition_id_tensor else None
    in_names, out_names, out_avals, zero_outs = [], [], [], []
    for alloc in nc.m.functions[0].allocations:
        if not isinstance(alloc, mybir.MemoryLocationSet):
            continue
        name = alloc.memorylocations[0].name
        if alloc.kind == 'ExternalInput':
            if name != partition_name:
                in_names.append(name)
        elif alloc.kind == 'ExternalOutput':
            shape = tuple(alloc.tensor_shape)
            dtype = mybir.dt.np(alloc.dtype)
            out_names.append(name)
            out_avals.append(jax.core.ShapedArray(shape, dtype))
            zero_outs.append(np.zeros(shape, dtype))
    n_params = len(in_names)
    n_outs = len(out_avals)
    all_in = list(in_names) + list(out_names)
    if partition_name is not None:
        all_in.append(partition_name)

    def _body(*args):
        operands = list(args)
        if partition_name is not None:
            operands.append(bass2jax.partition_id_tensor())
        outs = bass2jax._bass_exec_p.bind(
            *operands, out_avals=tuple(out_avals), in_names=tuple(all_in),
            out_names=tuple(out_names), lowering_input_output_aliases=(),
            sim_require_finite=True, sim_require_nnan=True, nc=nc)
        return tuple(outs)

    devices = jax.devices()[:n_cores]
    mesh = Mesh(np.asarray(devices), ('core',))
    in_specs = (PartitionSpec('core'),) * (n_params + n_outs)
    out_specs = (PartitionSpec('core'),) * n_outs
    sharded = jax.jit(shard_map(_body, mesh=mesh, in_specs=in_specs,
                                out_specs=out_specs, check_rep=False),
                      keep_unused=True)

    class R:
        pass
    r = R()
    r.in_names, r.out_names, r.out_avals = in_names, out_names, out_avals
    r.zero_outs, r.n_cores, r.sharded = zero_outs, n_cores, sharded
    return r


def put_inputs(r, in_maps):
    n = r.n_cores
    concat = [np.concatenate([np.asarray(in_maps[c][nm]) for c in range(n)],
                             axis=0)
              for nm in r.in_names]
    concat += [np.zeros((n * z.shape[0], *z.shape[1:]), z.dtype)
               for z in r.zero_outs]
    return [jax.device_put(a) for a in concat]


def execute(r, dev_args):
    outs = r.sharded(*dev_args)
    jax.block_until_ready(outs)
    return outs


def results_list(r, outs):
    res = []
    for c in range(r.n_cores):
        d = {}
        for i, nm in enumerate(r.out_names):
            full = np.asarray(outs[i])
            per = full.reshape(r.n_cores, *r.out_avals[i].shape)
            d[nm] = per[c]
        res.append(d)
    return res


# ---------------- public entry ----------------
_CACHE = {}


def _get_runner():
    if 'r' not in _CACHE:
        nc = build_kernel()
        nc.finalize()
        _CACHE['nc'] = nc
        _CACHE['r'] = make_runner(nc, NCORES)
    return _CACHE['r']


def kernel(query, memory, attention_weights, Wq, Wk, Wv, top_k):
    query = np.asarray(query, np.float32)
    memory = np.asarray(memory, np.float32)
    attention_weights = np.asarray(attention_weights, np.float32)
    Wq = np.asarray(Wq, np.float32)
    Wk = np.asarray(Wk, np.float32)
    Wv = np.asarray(Wv, np.float32)
    top_k = int(top_k)
    assert memory.shape == (N, D) and query.shape == (B, D)
    r = _get_runner()
    in_maps, scale, zp, qk = prep_inputs(query, memory, attention_weights,
                                         Wq, Wk, Wv)
    dev = put_inputs(r, in_maps)
    outs = execute(r, dev)
    res = results_list(r, outs)
    return host_tail(res, memory, attention_weights, Wv, scale, zp, qk,
                     top_k)


def kernel_timed(inputs, n_rep=10):
    """Returns (out, per-exec wallclock list in us)."""
    import time
    r = _get_runner()
    in_maps, scale, zp, qk = prep_inputs(
        np.asarray(inputs['query'], np.float32),
        np.asarray(inputs['memory'], np.float32),
        np.asarray(inputs['attention_weights'], np.float32),
        np.asarray(inputs['Wq'], np.float32),
        np.asarray(inputs['Wk'], np.float32),
        np.asarray(inputs['Wv'], np.float32))
    dev = put_inputs(r, in_maps)
    outs = execute(r, dev)
    ts = []
    for _ in range(n_rep):
        t0 = time.perf_counter()
        outs = execute(r, dev)
        ts.append((time.perf_counter() - t0) * 1e6)
    res = results_list(r, outs)
    out = host_tail(res, np.asarray(inputs['memory'], np.float32),
                    np.asarray(inputs['attention_weights'], np.float32),
                    np.asarray(inputs['Wv'], np.float32), scale, zp, qk,
                    top_k=int(inputs['top_k']))
    return out, ts


# revision 15
# speedup vs baseline: 2.1352x; 2.1352x over previous
"""Sharded retrieval-KNN kernel for Trainium2 (8 NeuronCores).

Self-contained: kernel(**inputs) -> np.ndarray [64, 64].

v4 design (v1 ~103us, v3 ~99us; op-level rates hardware-measured):
 - fp8 e4m3 code stream (codes centered by -128; attention weight cast
   directly) fed straight to the PE (fp8 moving x f16 stationary), so
   ACT does NO dtype conversion. Host re-score is bit-exact; fp8 only
   perturbs CANDIDATE selection (validated margin: worst bucket rank 2
   of the top-8-per-window budget).
 - whole code table SBUF-resident (61KB/partition); all stream DMAs
   issued up-front so PE never waits on buffer recycling.
 - PSUM drain split across both PSUM-capable engines at measured rates
   (DVE 1.042ns/elem + 130ns/instr; ACT copy 0.833ns/elem + 293ns;
   DVE f16 tensor_tensor max in 2x_1p mode 0.521ns/out; tensor_reduce
   has NO fast mode):
     A-chunks: DVE tensor_reduce max PSUM->pm (consecutive 64-col
       buckets);
     B-chunks: ACT copies PSUM->f16 staging; regions of 8/4/2/1
       B-chunks (not necessarily source-consecutive) fold INCREMENTALLY
       on DVE: each pair of staged chunks is folded as soon as copied
       (binary-counter merge), then the final 2048-wide running max is
       halved down to the region's nb buckets. Region buckets are the
       mod-nb staged-position classes, so every fold distance (2048,
       1024, ..., nb) preserves bucket membership; power-of-2 region
       sizes make chunk-pairwise folding legal.
 - A-chunks sit every ~4-5 chunks so DVE has filler while ACT copies,
   keeping the 2-slot PSUM recirculation from serializing; the stream
   ends on A-work + a tiny window-2 scan instead of a deep tree.
 - DVE Max8 + MaxIndex8 over 2 windows (704/273 buckets); [128, 8] u32
   bucket indices DMA'd out per window as soon as its scan finishes.

Host glue: exact fp32 re-score of expanded candidates (16 buckets x
64 slots x 2 parities x 8 cores per query), global top-k, softmax,
value projection.
"""

import sys
sys.path.insert(0, '/opt/trn_rl_repo')

import numpy as np
import ml_dtypes
import concourse.bass as bass
import concourse.mybir as mybir
from concourse import bacc, tile

F16 = mybir.dt.float16
F32 = mybir.dt.float32
F8 = mybir.dt.float8e4
U8 = mybir.dt.uint8
U32 = mybir.dt.uint32
ALU = mybir.AluOpType
AX = mybir.AxisListType

D = 64             # embedding dim
B = 64             # queries
NCORES = 8
N = 1_000_000
NSH = N // NCORES  # 125000 slots per core
NP = 125056        # padded slots per core (56 pad slots)
LANE = NP // 2     # 62528 per-parity lane columns
FOLD = 64          # cols per bucket
NB = LANE // FOLD  # 977 buckets per lane
NTOP = 8
NW = 2
NCAND = NW * NTOP

CW = 2048                      # chunk width (PSUM half)
WIDTHS = [512, 1536] + [CW] * 29 + [1088]
NCH = 32
A_SET = frozenset({0, 1, 6, 11, 16, 21, 26, 31})
REG_SIZES = [8, 8, 8]          # B-chunk regions, in order of appearance
# (start, len) DMA spans, ordered so each chunk's columns land before its
# processing position: the 1088 tail-columns chunk is processed 2nd (early
# DVE filler + shorter end-of-stream chain), so its span is issued 2nd.
DMA_SPANS = [(0, 512), (512, 1536), (2048, 2048), (4096, 4096),
             (8192, 4096), (12288, 8192), (20480, 8192), (28672, 8192),
             (36864, 8192), (45056, 8192), (53248, 8192), (61440, 1088)]
# c31 (1088 A-chunk) runs at position 30, before R2's final chunk: its
# DVE reduce hides under ACT's last copy instead of extending the tail
ORDER = list(range(30)) + [31, 30]
assert sum(l for _, l in DMA_SPANS) == LANE and sum(WIDTHS) == LANE
B_LIST = [i for i in range(NCH) if i not in A_SET]
assert sum(REG_SIZES) == len(B_LIST)
REGIONS = []
_i = 0
for _s in REG_SIZES:
    REGIONS.append(B_LIST[_i:_i + _s])
    _i += _s
for _r in REGIONS:
    assert all(WIDTHS[c] == CW for c in _r), 'region chunks must be 2048'

# segment layout in COMPLETION order (A-chunk at its index; region at its
# last chunk): (kind, payload, q0, nb)
def _layout():
    segs = []
    q0 = 0
    done_at = []
    g0s = np.concatenate([[0], np.cumsum(WIDTHS)]).astype(np.int64)
    for ci in sorted(A_SET):
        done_at.append((ORDER.index(ci), ('A', ci)))
    for ri, r in enumerate(REGIONS):
        done_at.append((max(ORDER.index(c) for c in r), ('B', ri)))
    done_at.sort()
    for _, (kind, x) in done_at:
        if kind == 'A':
            nb = WIDTHS[x] // FOLD
            segs.append(('A', x, q0, nb))
        else:
            nb = sum(WIDTHS[c] for c in REGIONS[x]) // FOLD
            segs.append(('B', x, q0, nb))
        q0 += nb
    assert q0 == NB
    return segs, g0s


SEGS, G0S = _layout()
_w1 = max(q0 + nb for k, x, q0, nb in SEGS if q0 + nb <= 720)
WIN = (_w1, NB - _w1)
Q0_OF_A = {x: q0 for k, x, q0, nb in SEGS if k == 'A'}
Q0_OF_R = {x: q0 for k, x, q0, nb in SEGS if k == 'B'}


def bucket_columns():
    """[NB, FOLD] lane-column indices of each global bucket."""
    cols = np.zeros((NB, FOLD), np.int64)
    for kind, x, q0, nb in SEGS:
        if kind == 'A':
            g0 = G0S[x]
            for b in range(nb):
                cols[q0 + b] = g0 + FOLD * b + np.arange(FOLD)
        else:
            r = REGIONS[x]
            s = np.arange(len(r) * CW)
            piece = s // CW
            col = G0S[np.asarray(r)[piece]] + (s % CW)
            for b in range(nb):
                cols[q0 + b] = col[s % nb == b]
    return cols


def build_kernel():
    nc = bacc.Bacc("TRN2", target_bir_lowering=False, debug=False,
                   num_devices=NCORES)

    codesT = nc.dram_tensor('codesT', [128, LANE + 256], U8,
                            kind='ExternalInput')
    o_idx = nc.dram_tensor('o_idx', [128, NCAND], U32, kind='ExternalOutput')

    wb = [0, WIN[0], NB]

    with tile.TileContext(nc) as tc:
        with tc.tile_pool(name='persist', bufs=1) as pp:
            lhsT_raw = pp.tile([128, 256], U8)
            nc.scalar.dma_start(lhsT_raw[:, :], codesT[:, LANE:LANE + 256])
            lhsT_sb = lhsT_raw[:, :].bitcast(F16)

            codes = pp.tile([128, LANE], U8, tag='codes')
            for off, span in DMA_SPANS:
                nc.sync.dma_start(codes[:, off:off + span],
                                  codesT[:, off:off + span])

            pm = pp.tile([128, NB], F16, tag='pm')
            wmax = pp.tile([128, NCAND], F16, tag='wmax')
            widx = pp.tile([128, NCAND], U32, tag='widx')
            # incremental-fold scratch: pair outputs, merge ping-pong,
            # halving ladder (all f16, offsets 4B-aligned)
            scrP = pp.tile([128, 4096], F16, tag='scrP')
            scrM = pp.tile([128, 4096], F16, tag='scrM')
            scrQ = pp.tile([128, 2048], F16, tag='scrQ')
            scrH = pp.tile([128, 1024], F16, tag='scrH')
            scrH2 = pp.tile([128, 512], F16, tag='scrH2')

            done_q = [0]

            def scan_windows(done, prev):
                for w in range(NW):
                    if done >= wb[w + 1] and prev < wb[w + 1]:
                        nc.vector.max(out=wmax[:, w * 8:(w + 1) * 8],
                                      in_=pm[:, wb[w]:wb[w + 1]])
                        nc.vector.max_index(
                            out=widx[:, w * 8:(w + 1) * 8],
                            in_max=wmax[:, w * 8:(w + 1) * 8],
                            in_values=pm[:, wb[w]:wb[w + 1]])
                        nc.sync.dma_start(
                            o_idx[:, w * 8:(w + 1) * 8],
                            widx[:, w * 8:(w + 1) * 8])

            def add_buckets(nb):
                prev = done_q[0]
                done_q[0] = prev + nb
                scan_windows(done_q[0], prev)

            with tc.tile_pool(name='stage', bufs=2) as sp, \
                 tc.tile_pool(name='ps', bufs=1, space='PSUM') as xp:
                ps = xp.tile([128, 4096], F32)

                reg_of = {}
                for ri, r in enumerate(REGIONS):
                    for c in r:
                        reg_of[c] = ri
                rstate = {}        # ri -> dict(st, n, stack, n2, n4)

                def emit_merges(ri):
                    """binary-counter merges; stack holds (ap, vchunks)
                    pieces, each 2048 wide. v2 pieces live in scrP, v4 in
                    scrM (2 slots), v8 in scrQ — a merge never writes the
                    buffer it reads."""
                    st = rstate[ri]
                    stack = st['stack']
                    while len(stack) >= 2 and stack[-1][1] == stack[-2][1]:
                        bsrc, v = stack.pop()
                        asrc, _ = stack.pop()
                        if v == 1:
                            dst = scrP[:, (st['n2'] % 2) * CW:
                                       (st['n2'] % 2 + 1) * CW]
                            st['n2'] += 1
                        elif v == 2:
                            dst = scrM[:, (st['n4'] % 2) * CW:
                                       (st['n4'] % 2 + 1) * CW]
                            st['n4'] += 1
                        else:
                            dst = scrQ[:, :CW]
                        nc.vector.tensor_tensor(dst, asrc, bsrc, ALU.max)
                        stack.append((dst, v * 2))

                def finish_region(ri):
                    st = rstate[ri]
                    assert len(st['stack']) == 1, 'region size must be 2^k'
                    cur, _ = st['stack'].pop()
                    nb = sum(WIDTHS[c] for c in REGIONS[ri]) // FOLD
                    cw = CW
                    ladder = (scrH, scrH2)
                    li = 0
                    while cw // 2 > nb:
                        half = cw // 2
                        dst = ladder[li % 2][:, :half]
                        nc.vector.tensor_tensor(dst, cur[:, :half],
                                                cur[:, half:cw], ALU.max)
                        cur, cw = dst, half
                        li += 1
                    q0 = Q0_OF_R[ri]
                    nc.vector.tensor_tensor(pm[:, q0:q0 + nb], cur[:, :nb],
                                            cur[:, nb:cw], ALU.max)
                    add_buckets(nb)

                for pi, ci in enumerate(ORDER):
                    gw = WIDTHS[ci]
                    g0 = int(G0S[ci])
                    poff = (pi % 2) * CW
                    r0 = 0
                    while r0 < gw:
                        bw = min(512, gw - r0)
                        nc.tensor.matmul(
                            ps[:, poff + r0:poff + r0 + bw], lhsT_sb,
                            codes[:, g0 + r0:g0 + r0 + bw].bitcast(F8),
                            start=True, stop=True)
                        r0 += bw
                    if ci in A_SET:
                        q0 = Q0_OF_A[ci]
                        nc.vector.tensor_reduce(
                            pm[:, q0:q0 + gw // FOLD],
                            ps[:, poff:poff + gw].rearrange(
                                'p (q k) -> p q k', k=FOLD),
                            AX.X, ALU.max)
                        add_buckets(gw // FOLD)
                    else:
                        ri = reg_of[ci]
                        if ri not in rstate:
                            stt = sp.tile([128, 8 * CW], F16, tag='st')
                            rstate[ri] = dict(st=stt, n=0, stack=[],
                                              n2=0, n4=0)
                        st = rstate[ri]
                        soff = st['n'] * CW
                        nc.scalar.copy(st['st'][:, soff:soff + CW],
                                       ps[:, poff:poff + CW])
                        st['n'] += 1
                        st['stack'].append(
                            (st['st'][:, soff:soff + CW], 1))
                        emit_merges(ri)
                        if ci == REGIONS[ri][-1]:
                            finish_region(ri)
    return nc


# ---------------- host glue ----------------

def _quant_params(memory):
    mn = memory.min()
    mx = memory.max()
    scale = (mx - mn) / np.float32(255.0)
    zp = -mn / scale
    return np.float32(scale), np.float32(zp)


def prep_inputs(query, memory, attention_weights, Wq, Wk, Wv):
    scale, zp = _quant_params(memory)
    codes = np.rint(memory / scale + zp).astype(np.float32)    # [N, 64]
    v8 = (codes[:, :63] - np.float32(128.0)).astype(
        ml_dtypes.float8_e4m3)                                 # [N, 63]
    aw8 = attention_weights.astype(ml_dtypes.float8_e4m3)      # [N]

    q = query @ Wq.T
    qk = (q @ Wk) / np.float32(np.sqrt(D))                     # [B, D]
    qks16 = (scale * qk[:, :63]).astype(np.float16)            # [B, 63]
    L = np.zeros((128, 128), np.float16)
    L[0:63, 0:64] = qks16.T
    L[63, 0:64] = np.float16(1.0)
    L[64:127, 64:128] = qks16.T
    L[127, 64:128] = np.float16(1.0)

    in_maps = []
    for c in range(NCORES):
        r64 = np.zeros((NP, 64), np.uint8)
        r64[:NSH, :63] = v8[c * NSH:(c + 1) * NSH].view(np.uint8)
        r64[:NSH, 63] = aw8[c * NSH:(c + 1) * NSH].view(np.uint8)
        codesT_h = np.ascontiguousarray(np.concatenate(
            [r64.reshape(LANE, 2, 64).transpose(1, 2, 0).reshape(128, LANE),
             L.view(np.uint8)], axis=1))
        in_maps.append(dict(codesT=codesT_h))
    return in_maps, scale, zp, qk


def host_tail(results, memory, attention_weights, Wv, scale, zp, qk, top_k):
    aw = attention_weights
    wb = [0, WIN[0], NB]
    bcols = bucket_columns()                                   # [NB, 64]
    cand = [[] for _ in range(B)]
    for c, r in enumerate(results):
        widx = r['o_idx'].astype(np.int64)                     # [128, 16]
        for p in range(128):
            par = 1 if p >= 64 else 0
            q_ = p % 64
            buckets = np.concatenate(
                [widx[p, w * 8:(w + 1) * 8] + wb[w] for w in range(NW)])
            cols = bcols[buckets].ravel()
            sl = 2 * cols + par
            ok = sl < NSH
            if ok.any():
                cand[q_].extend((c * NSH + sl[ok]).tolist())
    out = np.zeros((B, D), np.float32)
    for b in range(B):
        cs = np.unique(np.array(cand[b], dtype=np.int64))
        mdq = (np.rint(memory[cs] / scale + zp) - zp) * scale
        ss = qk[b] @ mdq.T + aw[cs]
        k = min(int(top_k), len(cs))
        ti = np.argsort(-ss, kind='stable')[:k]
        ts = ss[ti]
        w_ = np.exp(ts - ts.max())
        w_ = (w_ / w_.sum()).astype(np.float32)
        vals = mdq[ti] @ Wv.T
        out[b] = w_ @ vals
    return out


# ---------------- PJRT runner ----------------

import jax
from jax.sharding import Mesh, PartitionSpec
from jax.experimental.shard_map import shard_map
from concourse import bass2jax


def make_runner(nc, n_cores=8):
    bass2jax.install_neuronx_cc_hook()
    partition_name = nc.partition_id_tensor.name if nc.partition_id_tensor else None
    in_names, out_names, out_avals, zero_outs = [], [], [], []
    for alloc in nc.m.functions[0].allocations:
        if not isinstance(alloc, mybir.MemoryLocationSet):
            continue
        name = alloc.memorylocations[0].name
        if alloc.kind == 'ExternalInput':
            if name != partition_name:
                in_names.append(name)
        elif alloc.kind == 'ExternalOutput':
            shape = tuple(alloc.tensor_shape)
            dtype = mybir.dt.np(alloc.dtype)
            out_names.append(name)
            out_avals.append(jax.core.ShapedArray(shape, dtype))
            zero_outs.append(np.zeros(shape, dtype))
    n_params = len(in_names)
    n_outs = len(out_avals)
    all_in = list(in_names) + list(out_names)
    if partition_name is not None:
        all_in.append(partition_name)

    def _body(*args):
        operands = list(args)
        if partition_name is not None:
            operands.append(bass2jax.partition_id_tensor())
        outs = bass2jax._bass_exec_p.bind(
            *operands, out_avals=tuple(out_avals), in_names=tuple(all_in),
            out_names=tuple(out_names), lowering_input_output_aliases=(),
            sim_require_finite=True, sim_require_nnan=True, nc=nc)
        return tuple(outs)

    devices = jax.devices()[:n_cores]
    mesh = Mesh(np.asarray(devices), ('core',))
    in_specs = (PartitionSpec('core'),) * (n_params + n_outs)
    out_specs = (PartitionSpec('core'),) * n_outs
    sharded = jax.jit(shard_map(_body, mesh=mesh, in_specs=in_specs,
                                out_specs=out_specs, check_rep=False),
                      keep_unused=True)

    class R:
        pass
    r = R()
    r.in_names, r.out_names, r.out_avals = in_names, out_names, out_avals
    r.zero_outs, r.n_cores, r.sharded = zero_outs, n_cores, sharded
    return r


def put_inputs(r, in_maps):
    n = r.n_cores
    concat = [np.concatenate([np.asarray(in_maps[c][nm]) for c in range(n)],
                             axis=0)
              for nm in r.in_names]
    concat += [np.zeros((n * z.shape[0], *z.shape[1:]), z.dtype)
               for z in r.zero_outs]
    return [jax.device_put(a) for a in concat]


def execute(r, dev_args):
    outs = r.sharded(*dev_args)
    jax.block_until_ready(outs)
    return outs


def results_list(r, outs):
    res = []
    for c in range(r.n_cores):
        d = {}
        for i, nm in enumerate(r.out_names):
            full = np.asarray(outs[i])
            per = full.reshape(r.n_cores, *r.out_avals[i].shape)
            d[nm] = per[c]
        res.append(d)
    return res


# ---------------- public entry ----------------
_CACHE = {}


def _get_runner():
    if 'r' not in _CACHE:
        nc = build_kernel()
        nc.finalize()
        _CACHE['nc'] = nc
        _CACHE['r'] = make_runner(nc, NCORES)
    return _CACHE['r']


def kernel(query, memory, attention_weights, Wq, Wk, Wv, top_k):
    query = np.asarray(query, np.float32)
    memory = np.asarray(memory, np.float32)
    attention_weights = np.asarray(attention_weights, np.float32)
    Wq = np.asarray(Wq, np.float32)
    Wk = np.asarray(Wk, np.float32)
    Wv = np.asarray(Wv, np.float32)
    top_k = int(top_k)
    assert memory.shape == (N, D) and query.shape == (B, D)
    r = _get_runner()
    in_maps, scale, zp, qk = prep_inputs(query, memory, attention_weights,
                                         Wq, Wk, Wv)
    dev = put_inputs(r, in_maps)
    outs = execute(r, dev)
    res = results_list(r, outs)
    return host_tail(res, memory, attention_weights, Wv, scale, zp, qk,
                     top_k)


def kernel_timed(inputs, n_rep=10):
    """Returns (out, per-exec wallclock list in us)."""
    import time
    r = _get_runner()
    in_maps, scale, zp, qk = prep_inputs(
        np.asarray(inputs['query'], np.float32),
        np.asarray(inputs['memory'], np.float32),
        np.asarray(inputs['attention_weights'], np.float32),
        np.asarray(inputs['Wq'], np.float32),
        np.asarray(inputs['Wk'], np.float32),
        np.asarray(inputs['Wv'], np.float32))
    dev = put_inputs(r, in_maps)
    outs = execute(r, dev)
    ts = []
    for _ in range(n_rep):
        t0 = time.perf_counter()
        outs = execute(r, dev)
        ts.append((time.perf_counter() - t0) * 1e6)
    res = results_list(r, outs)
    out = host_tail(res, np.asarray(inputs['memory'], np.float32),
                    np.asarray(inputs['attention_weights'], np.float32),
                    np.asarray(inputs['Wv'], np.float32), scale, zp, qk,
                    top_k=int(inputs['top_k']))
    return out, ts
